# revision 3
# baseline (speedup 1.0000x reference)
"""Sharded cosine-similarity kNN retrieval kernel for Trainium2 (Bass/Tile).

Problem: one query [D] against keys [N, D]; return actions[top_k indices of
cosine similarity].  N=100000, D=2048, A=7, top_k<=8.

Strategy (fp8 TensorEngine scan, DMA-roofline bound):
  - Shard keys row-wise across 8 NeuronCores (12544 rows/core).  Keys are
    downcast to fp8e4m3 on the host and pre-packed per core into
    [sb=4, ki=128, c=8, ko=2, j=3136] so that (a) one superblock is a
    single 6.4MB DMA with one fully-contiguous 49KB segment per SBUF
    partition (near-peak HBM efficiency), and (b) tiles stream straight
    into the PE array as the DoubleRow moving operand (contraction chunk
    d = c*256 + ki*2 + ko, K=256 per chunk, 2 fp8/lane/cycle).  fp8 halves
    HBM traffic vs fp16 (25.7 MB/core) and moves the dot-product math off
    the (previously bottleneck) VectorE onto the otherwise-idle TensorE.
  - Device: per superblock, matmuls q8^T[256,1] @ keysT[256,<=512]
    accumulate the 8 chunk partials into psum banks [1,<=512]; psum->sbuf
    copies on ScalarE/DVE, one 50KB dots DMA out at the end.
  - Host reduce: take the top CAND=1024 rows by fp8 dot (candidate margin
    ~20 sigma: fp8 dot err std ~1.7 vs rank8->rank1024 dot gap ~50),
    re-score exactly in fp32 (cosine with norms), pick top_k with
    jax.lax.top_k tie semantics.  The device performs the full O(N*D)
    scan; the host only reduces candidates (O(CAND*D)).
"""

import sys

for _p in ("/opt/trn_rl_repo", "/opt/trn_rl_repo/concourse"):
    if _p not in sys.path:
        sys.path.insert(0, _p)

import numpy as np
import ml_dtypes

import concourse.bacc as bacc
from concourse import mybir
from concourse.bass import MemorySpace
from concourse.bass_utils import run_bass_kernel_spmd
from concourse.tile import TileContext

N, D, A = 100000, 2048, 7
EPS = 1e-8
N_CORES = 8
RPC = 12544                  # rows per core; 8*12544 = 100352 >= N
CHUNKS = 8                   # D split into 8 chunks of 256 (DoubleRow K)
NSB = 512                    # max rows per matmul / psum bank
SUPER = 3136                 # rows per superblock; 4 equal superblocks
SB_COUNT = RPC // SUPER      # 4
KBUFS = 3                    # superblock tile ring (49KB/partition each)
CAND = 1024                  # host re-score candidate count
F8 = ml_dtypes.float8_e4m3   # == TRN float8e4 (max 240, inf at S.1111.000)

_CACHE = {}


def _build_bass(repeats: int = 1, kbufs: int = KBUFS):
    """Build the per-core Bass program.

    repeats>1 wraps the streaming loop in a hardware For loop that re-reads
    the same DRAM shard; used only for wall-clock HW timing (slope over
    repeats cancels host/axon dispatch overhead)."""
    nc = bacc.Bacc(
        "TRN2",
        target_bir_lowering=False,
        debug=False,
        enable_asserts=False,
        num_devices=N_CORES,
    )
    f32 = mybir.dt.float32
    f8 = mybir.dt.float8e4
    keys_d = nc.dram_tensor(
        "keys8", [SB_COUNT, 128, CHUNKS, 2, SUPER], f8, kind="ExternalInput"
    ).ap()
    q_d = nc.dram_tensor("q8", [128, 2, 16], f8, kind="ExternalInput").ap()
    dots_d = nc.dram_tensor("dots", [1, RPC], f32, kind="ExternalOutput").ap()

    nb = (SUPER + NSB - 1) // NSB          # psum banks per superblock (7)

    with TileContext(nc) as tc:
        with tc.tile_pool(name="kpool", bufs=kbufs) as kpool, \
             tc.tile_pool(name="cpool", bufs=1) as cpool, \
             tc.tile_pool(name="ppool", bufs=8, space=MemorySpace.PSUM) as ppool:
            q_t = cpool.tile([128, 2, 16], f8)
            nc.sync.dma_start(out=q_t, in_=q_d)
            dots_t = cpool.tile([1, RPC], f32)

            def body():
                for sb in range(SB_COUNT):
                    kt = kpool.tile([128, CHUNKS, 2, SUPER], f8, tag="kt",
                                    name="kt")
                    nc.sync.dma_start(out=kt, in_=keys_d[sb])
                    pts = [ppool.tile([128, NSB], f32, tag="pt", name="pt")
                           for _ in range(nb)]
                    for c in range(CHUNKS):
                        for b in range(nb):
                            ncols = min(NSB, SUPER - b * NSB)
                            nc.tensor.matmul(
                                pts[b][0:1, :ncols],
                                q_t[:, :, c:c + 1],
                                kt[:, c, :, b * NSB:b * NSB + ncols],
                                start=(c == 0),
                                stop=(c == CHUNKS - 1),
                                perf_mode=mybir.MatmulPerfMode.DoubleRow,
                            )
                    for b in range(nb):
                        ncols = min(NSB, SUPER - b * NSB)
                        off = sb * SUPER + b * NSB
                        nc.any.tensor_copy(
                            dots_t[:, off:off + ncols], pts[b][0:1, :ncols])

            if repeats == 1:
                body()
            else:
                with tc.For_i(0, repeats, 1):
                    body()

            nc.sync.dma_start(out=dots_d, in_=dots_t)
    nc.compile()
    return nc


def _get_nc(repeats: int = 1, **kw):
    key = ("nc", repeats, tuple(sorted(kw.items())))
    if key not in _CACHE:
        _CACHE[key] = _build_bass(repeats, **kw)
    return _CACHE[key]


def _pack_keys_shard(keys8_shard: np.ndarray) -> np.ndarray:
    """[rows<=RPC, D] fp8 -> [SB, 128, CHUNKS, 2, SUPER].

    d = c*256 + ki*2 + ko; row = sb*SUPER + j."""
    rows = keys8_shard.shape[0]
    if rows < RPC:
        pad = np.zeros((RPC, D), dtype=F8)
        pad[:rows] = keys8_shard
        keys8_shard = pad
    # 2-D byte transpose (fast path), then split/permute
    t = np.ascontiguousarray(keys8_shard.view(np.uint8).T)   # [D, RPC]
    v = t.view(F8).reshape(CHUNKS, 128, 2, SB_COUNT, SUPER)  # c ki ko sb j
    return np.ascontiguousarray(v.transpose(3, 1, 0, 2, 4))  # sb ki c ko j


def _make_in_maps(keys: np.ndarray, query: np.ndarray):
    q8 = query.astype(F8)
    qarr = np.zeros((128, 2, 16), dtype=F8)
    qarr[:, :, :CHUNKS] = q8.reshape(CHUNKS, 128, 2).transpose(1, 2, 0)
    keys8 = keys.astype(F8)
    in_maps = []
    for i in range(N_CORES):
        lo, hi = i * RPC, min((i + 1) * RPC, N)
        in_maps.append({"keys8": _pack_keys_shard(keys8[lo:hi]), "q8": qarr})
    return in_maps


def _run_device(keys: np.ndarray, query: np.ndarray, trace: bool = False):
    """Run the SPMD kernel; returns (dots[8*RPC] fp8-precision, results)."""
    nc = _get_nc()
    in_maps = _make_in_maps(keys, query)
    res = run_bass_kernel_spmd(
        nc, in_maps, core_ids=list(range(N_CORES)), trace=trace
    )
    dots = np.concatenate(
        [out["dots"][0] for out in res.results])
    return dots, res


def kernel(**inputs) -> np.ndarray:
    query = np.asarray(inputs["query_key"], dtype=np.float32)
    keys = np.asarray(inputs["keys"], dtype=np.float32)
    actions = np.asarray(inputs["actions"])
    top_k = int(inputs["top_k"])
    if top_k <= 0:
        return actions[:0]
    top_k = min(top_k, keys.shape[0])

    dots8, _ = _run_device(keys, query)
    dots8 = dots8[:N]

    # candidate selection by fp8 dot, then exact fp32 cosine re-score
    m = min(max(CAND, 4 * top_k), N)
    cand = np.argpartition(-dots8, m - 1)[:m]
    kc = keys[cand]
    d_ex = kc @ query
    n_ex = np.sqrt((kc * kc).sum(axis=1))
    q_norm = np.float32(np.linalg.norm(query))
    sims_c = d_ex / np.maximum(n_ex * q_norm, np.float32(EPS))

    # top_k among candidates, ties to the lower index (jax.lax.top_k)
    order = np.lexsort((cand, -sims_c))
    idx = cand[order[:top_k]]
    return actions[idx]


# revision 5
# speedup vs baseline: 1.1108x; 1.1108x over previous
"""Sharded cosine-similarity kNN retrieval kernel for Trainium2 (Bass/Tile).

Problem: one query [D] against keys [N, D]; return actions[top_k indices of
cosine similarity].  N=100000, D=2048, A=7, top_k<=8.

Strategy (fp8 TensorEngine scan, DMA-roofline bound):
  - Shard keys row-wise across 8 NeuronCores (12544 rows/core).  Keys are
    downcast to fp8e4m3 on the host and pre-packed per core into
    [sb=4, c=8, ki=128, ko=2, j=3125] so that (a) each (superblock,
    d-chunk) is one 784KB DMA with one fully-contiguous 6.1KB segment per
    SBUF partition (many DMAs in flight saturate HBM), and (b) tiles
    stream straight into the PE array as the DoubleRow moving operand
    (contraction chunk d = c*256 + ki*2 + ko, 2 fp8/lane/cycle).  fp8 halves
    HBM traffic vs fp16 (25.7 MB/core) and moves the dot-product math off
    the (previously bottleneck) VectorE onto the otherwise-idle TensorE.
  - Device: per superblock, matmuls q8^T[256,1] @ keysT[256,<=512]
    accumulate the 8 chunk partials into psum banks [1,<=512]; psum->sbuf
    copies on ScalarE/DVE, one 50KB dots DMA out at the end.
  - Host reduce: take the top CAND=1024 rows by fp8 dot (candidate margin
    ~20 sigma: fp8 dot err std ~1.7 vs rank8->rank1024 dot gap ~50),
    re-score exactly in fp32 (cosine with norms), pick top_k with
    jax.lax.top_k tie semantics.  The device performs the full O(N*D)
    scan; the host only reduces candidates (O(CAND*D)).
"""

import sys

for _p in ("/opt/trn_rl_repo", "/opt/trn_rl_repo/concourse"):
    if _p not in sys.path:
        sys.path.insert(0, _p)

import numpy as np
import ml_dtypes

import concourse.bacc as bacc
from concourse import mybir
from concourse.bass import MemorySpace
from concourse.bass_utils import run_bass_kernel_spmd
from concourse.tile import TileContext

N, D, A = 100000, 2048, 7
EPS = 1e-8
N_CORES = 8
RPC = 12500                  # rows per core; 8*12500 = N exactly
CHUNKS = 8                   # D split into 8 chunks of 256 (DoubleRow K)
NSB = 512                    # max rows per matmul / psum bank
SUPER = 3125                 # rows per superblock; 4 equal superblocks
SB_COUNT = RPC // SUPER      # 4
KBUFS = 24                   # chunk tile ring (6.1KB/partition each)
DMA_SPLIT = 1                # DMAs per (superblock, chunk)
CAND = 1024                  # host re-score candidate count
F8 = ml_dtypes.float8_e4m3   # == TRN float8e4 (max 240, inf at S.1111.000)

_CACHE = {}


def _build_bass(repeats: int = 1, kbufs: int = KBUFS,
                dma_split: int = DMA_SPLIT):
    """Build the per-core Bass program.

    repeats>1 wraps the streaming loop in a hardware For loop that re-reads
    the same DRAM shard; used only for wall-clock HW timing (slope over
    repeats cancels host/axon dispatch overhead)."""
    nc = bacc.Bacc(
        "TRN2",
        target_bir_lowering=False,
        debug=False,
        enable_asserts=False,
        num_devices=N_CORES,
    )
    f32 = mybir.dt.float32
    f8 = mybir.dt.float8e4
    keys_d = nc.dram_tensor(
        "keys8", [SB_COUNT, CHUNKS, 128, 2, SUPER], f8, kind="ExternalInput"
    ).ap()
    q_d = nc.dram_tensor("q8", [128, 2, 16], f8, kind="ExternalInput").ap()
    dots_d = nc.dram_tensor("dots", [1, RPC], f32, kind="ExternalOutput").ap()

    nb = (SUPER + NSB - 1) // NSB          # psum banks per superblock (7)

    with TileContext(nc) as tc:
        with tc.tile_pool(name="kpool", bufs=kbufs) as kpool, \
             tc.tile_pool(name="cpool", bufs=1) as cpool, \
             tc.tile_pool(name="ppool", bufs=8, space=MemorySpace.PSUM) as ppool:
            q_t = cpool.tile([128, 2, 16], f8)
            nc.sync.dma_start(out=q_t, in_=q_d)
            dots_t = cpool.tile([1, RPC], f32)

            def body():
                for sb in range(SB_COUNT):
                    kts = []
                    for c in range(CHUNKS):
                        kt = kpool.tile([128, 2, SUPER], f8, tag="kt",
                                        name="kt")
                        if dma_split == 1:
                            nc.sync.dma_start(out=kt, in_=keys_d[sb, c])
                        else:
                            step = SUPER // dma_split
                            for s in range(dma_split):
                                j0 = s * step
                                j1 = SUPER if s == dma_split - 1 else j0 + step
                                nc.sync.dma_start(
                                    out=kt[:, :, j0:j1],
                                    in_=keys_d[sb, c, :, :, j0:j1])
                        kts.append(kt)
                    pts = [ppool.tile([128, NSB], f32, tag="pt", name="pt")
                           for _ in range(nb)]
                    for c in range(CHUNKS):
                        for b in range(nb):
                            ncols = min(NSB, SUPER - b * NSB)
                            nc.tensor.matmul(
                                pts[b][0:1, :ncols],
                                q_t[:, :, c:c + 1],
                                kts[c][:, :, b * NSB:b * NSB + ncols],
                                start=(c == 0),
                                stop=(c == CHUNKS - 1),
                                perf_mode=mybir.MatmulPerfMode.DoubleRow,
                            )
                    for b in range(nb):
                        ncols = min(NSB, SUPER - b * NSB)
                        off = sb * SUPER + b * NSB
                        nc.any.tensor_copy(
                            dots_t[:, off:off + ncols], pts[b][0:1, :ncols])

            if repeats == 1:
                body()
            else:
                with tc.For_i(0, repeats, 1):
                    body()

            nc.sync.dma_start(out=dots_d, in_=dots_t)
    nc.compile()
    return nc


def _get_nc(repeats: int = 1, **kw):
    key = ("nc", repeats, tuple(sorted(kw.items())))
    if key not in _CACHE:
        _CACHE[key] = _build_bass(repeats, **kw)
    return _CACHE[key]


def _pack_keys_shard(keys8_shard: np.ndarray) -> np.ndarray:
    """[rows<=RPC, D] fp8 -> [SB, CHUNKS, 128, 2, SUPER].

    d = c*256 + ki*2 + ko; row = sb*SUPER + j."""
    rows = keys8_shard.shape[0]
    if rows < RPC:
        pad = np.zeros((RPC, D), dtype=F8)
        pad[:rows] = keys8_shard
        keys8_shard = pad
    # 2-D byte transpose (fast path), then split/permute
    t = np.ascontiguousarray(keys8_shard.view(np.uint8).T)   # [D, RPC]
    v = t.view(F8).reshape(CHUNKS, 128, 2, SB_COUNT, SUPER)  # c ki ko sb j
    return np.ascontiguousarray(v.transpose(3, 0, 1, 2, 4))  # sb c ki ko j


def _make_in_maps(keys: np.ndarray, query: np.ndarray):
    q8 = query.astype(F8)
    qarr = np.zeros((128, 2, 16), dtype=F8)
    qarr[:, :, :CHUNKS] = q8.reshape(CHUNKS, 128, 2).transpose(1, 2, 0)
    keys8 = keys.astype(F8)
    in_maps = []
    for i in range(N_CORES):
        lo, hi = i * RPC, min((i + 1) * RPC, N)
        in_maps.append({"keys8": _pack_keys_shard(keys8[lo:hi]), "q8": qarr})
    return in_maps


def _run_device(keys: np.ndarray, query: np.ndarray, trace: bool = False):
    """Run the SPMD kernel; returns (dots[8*RPC] fp8-precision, results)."""
    nc = _get_nc()
    in_maps = _make_in_maps(keys, query)
    res = run_bass_kernel_spmd(
        nc, in_maps, core_ids=list(range(N_CORES)), trace=trace
    )
    dots = np.concatenate(
        [out["dots"][0] for out in res.results])
    return dots, res


def kernel(**inputs) -> np.ndarray:
    query = np.asarray(inputs["query_key"], dtype=np.float32)
    keys = np.asarray(inputs["keys"], dtype=np.float32)
    actions = np.asarray(inputs["actions"])
    top_k = int(inputs["top_k"])
    if top_k <= 0:
        return actions[:0]
    top_k = min(top_k, keys.shape[0])

    dots8, _ = _run_device(keys, query)
    dots8 = dots8[:N]

    # candidate selection by fp8 dot, then exact fp32 cosine re-score
    m = min(max(CAND, 4 * top_k), N)
    cand = np.argpartition(-dots8, m - 1)[:m]
    kc = keys[cand]
    d_ex = kc @ query
    n_ex = np.sqrt((kc * kc).sum(axis=1))
    q_norm = np.float32(np.linalg.norm(query))
    sims_c = d_ex / np.maximum(n_ex * q_norm, np.float32(EPS))

    # top_k among candidates, ties to the lower index (jax.lax.top_k)
    order = np.lexsort((cand, -sims_c))
    idx = cand[order[:top_k]]
    return actions[idx]


# revision 8
# speedup vs baseline: 1.1155x; 1.0042x over previous
"""Sharded cosine-similarity kNN retrieval kernel for Trainium2 (Bass/Tile).

Problem: one query [D] against keys [N, D]; return actions[top_k indices of
cosine similarity].  N=100000, D=2048, A=7, top_k<=8.

Strategy (fp8 TensorEngine scan, DMA-roofline bound):
  - Shard keys row-wise across 8 NeuronCores (12544 rows/core).  Keys are
    downcast to fp8e4m3 on the host and pre-packed per core into
    [sb=4, c=8, ki=128, ko=2, j=3125] so that (a) each (superblock,
    d-chunk) is one 784KB DMA with one fully-contiguous 6.1KB segment per
    SBUF partition (many DMAs in flight saturate HBM), and (b) tiles
    stream straight into the PE array as the DoubleRow moving operand
    (contraction chunk d = c*256 + ki*2 + ko, 2 fp8/lane/cycle).  fp8 halves
    HBM traffic vs fp16 (25.7 MB/core) and moves the dot-product math off
    the (previously bottleneck) VectorE onto the otherwise-idle TensorE.
  - Device: per superblock, matmuls q8^T[256,1] @ keysT[256,<=512]
    accumulate the 8 chunk partials into psum banks [1,<=512]; psum->sbuf
    copies on ScalarE/DVE, one 50KB dots DMA out at the end.
  - Host reduce: take the top CAND=1024 rows by fp8 dot (candidate margin
    ~20 sigma: fp8 dot err std ~1.7 vs rank8->rank1024 dot gap ~50),
    re-score exactly in fp32 (cosine with norms), pick top_k with
    jax.lax.top_k tie semantics.  The device performs the full O(N*D)
    scan; the host only reduces candidates (O(CAND*D)).
"""

import sys

for _p in ("/opt/trn_rl_repo", "/opt/trn_rl_repo/concourse"):
    if _p not in sys.path:
        sys.path.insert(0, _p)

import numpy as np
import ml_dtypes

import concourse.bacc as bacc
from concourse import mybir
from concourse.bass import MemorySpace
from concourse.bass_utils import run_bass_kernel_spmd
from concourse.tile import TileContext

N, D, A = 100000, 2048, 7
EPS = 1e-8
N_CORES = 8
RPC = 12500                  # rows per core; 8*12500 = N exactly
CHUNKS = 8                   # D split into 8 chunks of 256 (DoubleRow K)
NSB = 512                    # max rows per matmul / psum bank
SUPER = 3125                 # rows per superblock; 4 equal superblocks
SB_COUNT = RPC // SUPER      # 4
KBUFS = 24                   # chunk tile ring (6.1KB/partition each)
DMA_SPLIT = 1                # DMAs per (superblock, chunk)
CAND = 1024                  # host re-score candidate count
F8 = ml_dtypes.float8_e4m3   # == TRN float8e4 (max 240, inf at S.1111.000)

_CACHE = {}


def _build_bass(repeats: int = 1, kbufs: int = KBUFS,
                dma_split: int = DMA_SPLIT):
    """Build the per-core Bass program.

    repeats>1 wraps the streaming loop in a hardware For loop that re-reads
    the same DRAM shard; used only for wall-clock HW timing (slope over
    repeats cancels host/axon dispatch overhead)."""
    nc = bacc.Bacc(
        "TRN2",
        target_bir_lowering=False,
        debug=False,
        enable_asserts=False,
        num_devices=N_CORES,
    )
    f32 = mybir.dt.float32
    f8 = mybir.dt.float8e4
    keys_d = nc.dram_tensor(
        "keys8", [SB_COUNT, CHUNKS, 128, 2, SUPER], f8, kind="ExternalInput"
    ).ap()
    q_d = nc.dram_tensor("q8", [128, 2, 16], f8, kind="ExternalInput").ap()
    dots_d = nc.dram_tensor("dots", [1, RPC], f32, kind="ExternalOutput").ap()

    nb = (SUPER + NSB - 1) // NSB          # psum banks per superblock (7)

    with TileContext(nc) as tc:
        with tc.tile_pool(name="kpool", bufs=kbufs) as kpool, \
             tc.tile_pool(name="cpool", bufs=1) as cpool, \
             tc.tile_pool(name="ppool", bufs=8, space=MemorySpace.PSUM) as ppool:
            q_t = cpool.tile([128, 2, 16], f8)
            nc.sync.dma_start(out=q_t, in_=q_d)
            dots_t = cpool.tile([1, RPC], f32)

            def body():
                for sb in range(SB_COUNT):
                    kts = []
                    for c in range(CHUNKS):
                        kt = kpool.tile([128, 2, SUPER], f8, tag="kt",
                                        name="kt")
                        if dma_split == 1:
                            nc.sync.dma_start(out=kt, in_=keys_d[sb, c])
                        else:
                            step = SUPER // dma_split
                            for s in range(dma_split):
                                j0 = s * step
                                j1 = SUPER if s == dma_split - 1 else j0 + step
                                nc.sync.dma_start(
                                    out=kt[:, :, j0:j1],
                                    in_=keys_d[sb, c, :, :, j0:j1])
                        kts.append(kt)
                    pts = [ppool.tile([128, NSB], f32, tag="pt", name="pt")
                           for _ in range(nb)]
                    for c in range(CHUNKS):
                        for b in range(nb):
                            ncols = min(NSB, SUPER - b * NSB)
                            nc.tensor.matmul(
                                pts[b][0:1, :ncols],
                                q_t[:, :, c:c + 1],
                                kts[c][:, :, b * NSB:b * NSB + ncols],
                                start=(c == 0),
                                stop=(c == CHUNKS - 1),
                                perf_mode=mybir.MatmulPerfMode.DoubleRow,
                            )
                    for b in range(nb):
                        ncols = min(NSB, SUPER - b * NSB)
                        off = sb * SUPER + b * NSB
                        nc.any.tensor_copy(
                            dots_t[:, off:off + ncols], pts[b][0:1, :ncols])

            if repeats == 1:
                body()
            else:
                with tc.For_i(0, repeats, 1):
                    body()

            nc.sync.dma_start(out=dots_d, in_=dots_t)
    nc.compile()
    return nc


def _get_nc(repeats: int = 1, **kw):
    key = ("nc", repeats, tuple(sorted(kw.items())))
    if key not in _CACHE:
        _CACHE[key] = _build_bass(repeats, **kw)
    return _CACHE[key]


def _pack_keys_shard(keys8_shard: np.ndarray) -> np.ndarray:
    """[rows<=RPC, D] fp8 -> [SB, CHUNKS, 128, 2, SUPER].

    d = c*256 + ki*2 + ko; row = sb*SUPER + j."""
    rows = keys8_shard.shape[0]
    if rows < RPC:
        pad = np.zeros((RPC, D), dtype=F8)
        pad[:rows] = keys8_shard
        keys8_shard = pad
    # 2-D byte transpose (fast path), then split/permute
    t = np.ascontiguousarray(keys8_shard.view(np.uint8).T)   # [D, RPC]
    v = t.view(F8).reshape(CHUNKS, 128, 2, SB_COUNT, SUPER)  # c ki ko sb j
    return np.ascontiguousarray(v.transpose(3, 0, 1, 2, 4))  # sb c ki ko j


def _make_in_maps(keys: np.ndarray, query: np.ndarray):
    q8 = query.astype(F8)
    qarr = np.zeros((128, 2, 16), dtype=F8)
    qarr[:, :, :CHUNKS] = q8.reshape(CHUNKS, 128, 2).transpose(1, 2, 0)
    keys8 = keys.astype(F8)
    in_maps = []
    for i in range(N_CORES):
        lo, hi = i * RPC, min((i + 1) * RPC, N)
        in_maps.append({"keys8": _pack_keys_shard(keys8[lo:hi]), "q8": qarr})
    return in_maps


def _run_device(keys: np.ndarray, query: np.ndarray, trace: bool = False):
    """Run the SPMD kernel; returns (dots[8*RPC] fp8-precision, results)."""
    nc = _get_nc()
    in_maps = _make_in_maps(keys, query)
    res = run_bass_kernel_spmd(
        nc, in_maps, core_ids=list(range(N_CORES)), trace=trace
    )
    dots = np.concatenate(
        [out["dots"][0] for out in res.results])
    return dots, res


def _host_topk(keys, query, actions, top_k):
    """Generic fallback (not used for the canonical problem shape)."""
    sims = (keys @ query) / np.maximum(
        np.linalg.norm(keys, axis=1) * np.float32(np.linalg.norm(query)),
        np.float32(EPS))
    cand = np.argpartition(-sims, top_k - 1)[:top_k]
    order = np.lexsort((cand, -sims[cand]))
    return actions[cand[order]]


def kernel(**inputs) -> np.ndarray:
    query = np.asarray(inputs["query_key"], dtype=np.float32)
    keys = np.asarray(inputs["keys"], dtype=np.float32)
    actions = np.asarray(inputs["actions"])
    top_k = int(inputs["top_k"])
    if top_k <= 0:
        return actions[:0]
    top_k = min(top_k, keys.shape[0])

    if keys.shape != (N, D) or query.shape != (D,):
        return _host_topk(keys, query, actions, top_k)

    dots8, _ = _run_device(keys, query)
    dots8 = dots8[:N]

    # candidate selection by fp8 dot, then exact fp32 cosine re-score
    m = min(max(CAND, 4 * top_k), N)
    cand = np.argpartition(-dots8, m - 1)[:m]
    kc = keys[cand]
    d_ex = kc @ query
    n_ex = np.sqrt((kc * kc).sum(axis=1))
    q_norm = np.float32(np.linalg.norm(query))
    sims_c = d_ex / np.maximum(n_ex * q_norm, np.float32(EPS))

    # top_k among candidates, ties to the lower index (jax.lax.top_k)
    order = np.lexsort((cand, -sims_c))
    idx = cand[order[:top_k]]
    return actions[idx]


# revision 9
# speedup vs baseline: 1.2649x; 1.1339x over previous
"""Sharded cosine-similarity kNN retrieval kernel for Trainium2 (Bass/Tile).

Problem: one query [D] against keys [N, D]; return actions[top_k indices of
cosine similarity].  N=100000, D=2048, A=7, top_k<=8.

Strategy (fp8 TensorEngine scan, DMA-roofline bound):
  - Shard keys row-wise across 8 NeuronCores (12544 rows/core).  Keys are
    downcast to fp8e4m3 on the host and pre-packed per core into
    [sb=4, c=8, ki=128, ko=2, j=3125] so that (a) each (superblock,
    d-chunk) is one 784KB DMA with one fully-contiguous 6.1KB segment per
    SBUF partition (many DMAs in flight saturate HBM), and (b) tiles
    stream straight into the PE array as the DoubleRow moving operand
    (contraction chunk d = c*256 + ki*2 + ko, 2 fp8/lane/cycle).  fp8 halves
    HBM traffic vs fp16 (25.7 MB/core) and moves the dot-product math off
    the (previously bottleneck) VectorE onto the otherwise-idle TensorE.
  - Device: per superblock, matmuls q8^T[256,1] @ keysT[256,<=512]
    accumulate the 8 chunk partials into psum banks [1,<=512]; psum->sbuf
    copies on ScalarE/DVE, one 50KB dots DMA out at the end.
  - Host reduce: take the top CAND=1024 rows by fp8 dot (candidate margin
    ~20 sigma: fp8 dot err std ~1.7 vs rank8->rank1024 dot gap ~50),
    re-score exactly in fp32 (cosine with norms), pick top_k with
    jax.lax.top_k tie semantics.  The device performs the full O(N*D)
    scan; the host only reduces candidates (O(CAND*D)).
"""

import sys

for _p in ("/opt/trn_rl_repo", "/opt/trn_rl_repo/concourse"):
    if _p not in sys.path:
        sys.path.insert(0, _p)

import numpy as np
import ml_dtypes

import concourse.bacc as bacc
from concourse import mybir
from concourse.bass import MemorySpace
from concourse.bass_utils import run_bass_kernel_spmd
from concourse.tile import TileContext

N, D, A = 100000, 2048, 7
EPS = 1e-8
N_CORES = 8
RPC = 12500                  # rows per core; 8*12500 = N exactly
CHUNKS = 8                   # D split into 8 chunks of 256 (DoubleRow K)
NSB = 512                    # max rows per matmul / psum bank
SUPER = 3125                 # rows per superblock; 4 equal superblocks
SB_COUNT = RPC // SUPER      # 4
KBUFS = 25                   # chunk tile ring (6.1KB/partition each)
DMA_SPLIT = 1                # DMAs per (superblock, chunk)
CAND = 1024                  # host re-score candidate count
F8 = ml_dtypes.float8_e4m3   # == TRN float8e4 (max 240, inf at S.1111.000)

_CACHE = {}


def _build_bass(repeats: int = 1, kbufs: int = KBUFS,
                dma_split: int = DMA_SPLIT):
    """Build the per-core Bass program.

    repeats>1 wraps the streaming loop in a hardware For loop that re-reads
    the same DRAM shard; used only for wall-clock HW timing (slope over
    repeats cancels host/axon dispatch overhead)."""
    nc = bacc.Bacc(
        "TRN2",
        target_bir_lowering=False,
        debug=False,
        enable_asserts=False,
        num_devices=N_CORES,
    )
    f32 = mybir.dt.float32
    f8 = mybir.dt.float8e4
    keys_d = nc.dram_tensor(
        "keys8", [SB_COUNT, CHUNKS, 128, 2, SUPER], f8, kind="ExternalInput"
    ).ap()
    q_d = nc.dram_tensor("q8", [128, 2, 16], f8, kind="ExternalInput").ap()
    dots_d = nc.dram_tensor("dots", [1, RPC], f32, kind="ExternalOutput").ap()

    nb = (SUPER + NSB - 1) // NSB          # psum banks per superblock (7)

    with TileContext(nc) as tc:
        with tc.tile_pool(name="kpool", bufs=kbufs) as kpool, \
             tc.tile_pool(name="cpool", bufs=1) as cpool, \
             tc.tile_pool(name="ppool", bufs=8, space=MemorySpace.PSUM) as ppool:
            q_t = cpool.tile([128, 2, 16], f8)
            nc.sync.dma_start(out=q_t, in_=q_d)
            dots_t = cpool.tile([1, RPC], f32)

            def body():
                for sb in range(SB_COUNT):
                    kts = []
                    for c in range(CHUNKS):
                        kt = kpool.tile([128, 2, SUPER], f8, tag="kt",
                                        name="kt")
                        if dma_split == 1:
                            nc.sync.dma_start(out=kt, in_=keys_d[sb, c])
                        else:
                            step = SUPER // dma_split
                            for s in range(dma_split):
                                j0 = s * step
                                j1 = SUPER if s == dma_split - 1 else j0 + step
                                nc.sync.dma_start(
                                    out=kt[:, :, j0:j1],
                                    in_=keys_d[sb, c, :, :, j0:j1])
                        kts.append(kt)
                    pts = [ppool.tile([128, NSB], f32, tag="pt", name="pt")
                           for _ in range(nb)]
                    for c in range(CHUNKS):
                        for b in range(nb):
                            ncols = min(NSB, SUPER - b * NSB)
                            nc.tensor.matmul(
                                pts[b][0:1, :ncols],
                                q_t[:, :, c:c + 1],
                                kts[c][:, :, b * NSB:b * NSB + ncols],
                                start=(c == 0),
                                stop=(c == CHUNKS - 1),
                                perf_mode=mybir.MatmulPerfMode.DoubleRow,
                            )
                    for b in range(nb):
                        ncols = min(NSB, SUPER - b * NSB)
                        off = sb * SUPER + b * NSB
                        nc.any.tensor_copy(
                            dots_t[:, off:off + ncols], pts[b][0:1, :ncols])

            if repeats == 1:
                body()
            else:
                with tc.For_i(0, repeats, 1):
                    body()

            nc.sync.dma_start(out=dots_d, in_=dots_t)
    nc.compile()
    return nc


def _get_nc(repeats: int = 1, **kw):
    key = ("nc", repeats, tuple(sorted(kw.items())))
    if key not in _CACHE:
        _CACHE[key] = _build_bass(repeats, **kw)
    return _CACHE[key]


def _pack_keys_shard(keys8_shard: np.ndarray) -> np.ndarray:
    """[rows<=RPC, D] fp8 -> [SB, CHUNKS, 128, 2, SUPER].

    d = c*256 + ki*2 + ko; row = sb*SUPER + j."""
    rows = keys8_shard.shape[0]
    if rows < RPC:
        pad = np.zeros((RPC, D), dtype=F8)
        pad[:rows] = keys8_shard
        keys8_shard = pad
    # 2-D byte transpose (fast path), then split/permute
    t = np.ascontiguousarray(keys8_shard.view(np.uint8).T)   # [D, RPC]
    v = t.view(F8).reshape(CHUNKS, 128, 2, SB_COUNT, SUPER)  # c ki ko sb j
    return np.ascontiguousarray(v.transpose(3, 0, 1, 2, 4))  # sb c ki ko j


def _make_in_maps(keys: np.ndarray, query: np.ndarray):
    q8 = query.astype(F8)
    qarr = np.zeros((128, 2, 16), dtype=F8)
    qarr[:, :, :CHUNKS] = q8.reshape(CHUNKS, 128, 2).transpose(1, 2, 0)
    keys8 = keys.astype(F8)
    in_maps = []
    for i in range(N_CORES):
        lo, hi = i * RPC, min((i + 1) * RPC, N)
        in_maps.append({"keys8": _pack_keys_shard(keys8[lo:hi]), "q8": qarr})
    return in_maps


def _run_device(keys: np.ndarray, query: np.ndarray, trace: bool = False):
    """Run the SPMD kernel; returns (dots[8*RPC] fp8-precision, results)."""
    nc = _get_nc()
    in_maps = _make_in_maps(keys, query)
    res = run_bass_kernel_spmd(
        nc, in_maps, core_ids=list(range(N_CORES)), trace=trace
    )
    dots = np.concatenate(
        [out["dots"][0] for out in res.results])
    return dots, res


def _host_topk(keys, query, actions, top_k):
    """Generic fallback (not used for the canonical problem shape)."""
    sims = (keys @ query) / np.maximum(
        np.linalg.norm(keys, axis=1) * np.float32(np.linalg.norm(query)),
        np.float32(EPS))
    cand = np.argpartition(-sims, top_k - 1)[:top_k]
    order = np.lexsort((cand, -sims[cand]))
    return actions[cand[order]]


def kernel(**inputs) -> np.ndarray:
    query = np.asarray(inputs["query_key"], dtype=np.float32)
    keys = np.asarray(inputs["keys"], dtype=np.float32)
    actions = np.asarray(inputs["actions"])
    top_k = int(inputs["top_k"])
    if top_k <= 0:
        return actions[:0]
    top_k = min(top_k, keys.shape[0])

    if keys.shape != (N, D) or query.shape != (D,):
        return _host_topk(keys, query, actions, top_k)

    dots8, _ = _run_device(keys, query)
    dots8 = dots8[:N]

    # candidate selection by fp8 dot, then exact fp32 cosine re-score
    m = min(max(CAND, 4 * top_k), N)
    cand = np.argpartition(-dots8, m - 1)[:m]
    kc = keys[cand]
    d_ex = kc @ query
    n_ex = np.sqrt((kc * kc).sum(axis=1))
    q_norm = np.float32(np.linalg.norm(query))
    sims_c = d_ex / np.maximum(n_ex * q_norm, np.float32(EPS))

    # top_k among candidates, ties to the lower index (jax.lax.top_k)
    order = np.lexsort((cand, -sims_c))
    idx = cand[order[:top_k]]
    return actions[idx]


# revision 10
# speedup vs baseline: 1.2741x; 1.0073x over previous
"""Two-pass sharded cosine-similarity kNN retrieval for Trainium2 (Bass/Tile).

Pass 1 scans chunks 0-5 (d<1536, 75% of bytes) of all N rows in fp8 on the
TensorEngine; the host screens to the top M1=16384 rows by partial dot
(empirically the true top-8 sit at partial rank <=87; ~5.4 sigma margin).
Pass 2 scans only the survivors' remaining 512 dims (2048 rows/core).
Host reduce: full fp8 dot = p1 + p2 for survivors, top-1024, exact fp32
cosine re-score, top_k with jax.lax.top_k tie semantics.

Device bytes/core: 19.2MB + 0.5MB vs 25.6MB single-pass (-22%).
"""

import sys

for _p in ("/opt/trn_rl_repo", "/opt/trn_rl_repo/concourse"):
    if _p not in sys.path:
        sys.path.insert(0, _p)

import numpy as np
import ml_dtypes

import concourse.bacc as bacc
from concourse import mybir
from concourse.bass import MemorySpace
from concourse.bass_utils import run_bass_kernel_spmd
from concourse.tile import TileContext

N, D, A = 100000, 2048, 7
EPS = 1e-8
N_CORES = 8
NSB = 512                    # max rows per matmul / psum bank
CAND = 1024                  # final exact re-score candidate count
F8 = ml_dtypes.float8_e4m3

# pass 1: chunks 0-5 over all rows
RPC1, SUPER1, LO1, HI1, KBUFS1 = 12500, 3125, 0, 6, 25
# pass 2: chunks 6-7 over M1 screened rows
M1 = 16384
RPC2, SUPER2, LO2, HI2, KBUFS2 = 2048, 1024, 6, 8, 8

_CACHE = {}


def _build_bass(repeats: int, lo: int, hi: int, rpc: int, sup: int,
                kbufs: int):
    """Per-core program: fp8 DoubleRow matvec over chunks [lo,hi) of rpc rows."""
    nc = bacc.Bacc(
        "TRN2",
        target_bir_lowering=False,
        debug=False,
        enable_asserts=False,
        num_devices=N_CORES,
    )
    f32 = mybir.dt.float32
    f8 = mybir.dt.float8e4
    nch = hi - lo
    sbc = rpc // sup
    keys_d = nc.dram_tensor(
        "keys8", [sbc, nch, 128, 2, sup], f8, kind="ExternalInput"
    ).ap()
    q_d = nc.dram_tensor("q8", [128, 2, 16], f8, kind="ExternalInput").ap()
    dots_d = nc.dram_tensor("dots", [1, rpc], f32, kind="ExternalOutput").ap()

    nb = (sup + NSB - 1) // NSB

    with TileContext(nc) as tc:
        with tc.tile_pool(name="kpool", bufs=kbufs) as kpool, \
             tc.tile_pool(name="cpool", bufs=1) as cpool, \
             tc.tile_pool(name="ppool", bufs=8, space=MemorySpace.PSUM) as ppool:
            q_t = cpool.tile([128, 2, 16], f8)
            nc.sync.dma_start(out=q_t, in_=q_d)
            dots_t = cpool.tile([1, rpc], f32)

            def body():
                for sb in range(sbc):
                    kts = []
                    for ci in range(nch):
                        kt = kpool.tile([128, 2, sup], f8, tag="kt",
                                        name="kt")
                        nc.sync.dma_start(out=kt, in_=keys_d[sb, ci])
                        kts.append(kt)
                    pts = [ppool.tile([128, NSB], f32, tag="pt", name="pt")
                           for _ in range(nb)]
                    for ci in range(nch):
                        for b in range(nb):
                            ncols = min(NSB, sup - b * NSB)
                            nc.tensor.matmul(
                                pts[b][0:1, :ncols],
                                q_t[:, :, lo + ci:lo + ci + 1],
                                kts[ci][:, :, b * NSB:b * NSB + ncols],
                                start=(ci == 0),
                                stop=(ci == nch - 1),
                                perf_mode=mybir.MatmulPerfMode.DoubleRow,
                            )
                    for b in range(nb):
                        ncols = min(NSB, sup - b * NSB)
                        off = sb * sup + b * NSB
                        nc.any.tensor_copy(
                            dots_t[:, off:off + ncols], pts[b][0:1, :ncols])

            if repeats == 1:
                body()
            else:
                with tc.For_i(0, repeats, 1):
                    body()

            nc.sync.dma_start(out=dots_d, in_=dots_t)
    nc.compile()
    return nc


def _get_nc_p1(repeats: int = 1):
    key = ("p1", repeats)
    if key not in _CACHE:
        _CACHE[key] = _build_bass(repeats, LO1, HI1, RPC1, SUPER1, KBUFS1)
    return _CACHE[key]


def _get_nc_p2(repeats: int = 1):
    key = ("p2", repeats)
    if key not in _CACHE:
        _CACHE[key] = _build_bass(repeats, LO2, HI2, RPC2, SUPER2, KBUFS2)
    return _CACHE[key]


def _pack(shard8_t: np.ndarray, nch: int, sbc: int, sup: int) -> np.ndarray:
    """d-major fp8 bytes [nch*256, rows] -> [sbc, nch, 128, 2, sup]."""
    v = shard8_t.view(F8).reshape(nch, 128, 2, sbc, sup)      # c ki ko sb j
    return np.ascontiguousarray(v.transpose(3, 0, 1, 2, 4))   # sb c ki ko j


def _qarr(query: np.ndarray) -> np.ndarray:
    q8 = query.astype(F8)
    qa = np.zeros((128, 2, 16), dtype=F8)
    qa[:, :, :8] = q8.reshape(8, 128, 2).transpose(1, 2, 0)
    return qa


def _make_in_maps_p1(keys8: np.ndarray, qa: np.ndarray):
    in_maps = []
    for i in range(N_CORES):
        sh = keys8[i * RPC1:(i + 1) * RPC1]                   # [RPC1, D]
        t = np.ascontiguousarray(sh.view(np.uint8).T)          # [D, RPC1]
        in_maps.append(
            {"keys8": _pack(t[:HI1 * 256], HI1 - LO1, RPC1 // SUPER1, SUPER1),
             "q8": qa})
    return in_maps


def _make_in_maps_p2(keys8: np.ndarray, qa: np.ndarray, cand: np.ndarray):
    in_maps = []
    for i in range(N_CORES):
        rows = keys8[cand[i * RPC2:(i + 1) * RPC2], LO2 * 256:]  # [RPC2, 512]
        t = np.ascontiguousarray(rows.view(np.uint8).T)           # [512, RPC2]
        in_maps.append(
            {"keys8": _pack(t, HI2 - LO2, RPC2 // SUPER2, SUPER2), "q8": qa})
    return in_maps


def _run(nc, in_maps):
    res = run_bass_kernel_spmd(
        nc, in_maps, core_ids=list(range(N_CORES)), trace=False)
    return np.concatenate([out["dots"][0] for out in res.results])


def _host_topk(keys, query, actions, top_k):
    """Generic fallback (not used for the canonical problem shape)."""
    sims = (keys @ query) / np.maximum(
        np.linalg.norm(keys, axis=1) * np.float32(np.linalg.norm(query)),
        np.float32(EPS))
    cand = np.argpartition(-sims, top_k - 1)[:top_k]
    order = np.lexsort((cand, -sims[cand]))
    return actions[cand[order]]


def kernel(**inputs) -> np.ndarray:
    query = np.asarray(inputs["query_key"], dtype=np.float32)
    keys = np.asarray(inputs["keys"], dtype=np.float32)
    actions = np.asarray(inputs["actions"])
    top_k = int(inputs["top_k"])
    if top_k <= 0:
        return actions[:0]
    top_k = min(top_k, keys.shape[0])

    if keys.shape != (N, D) or query.shape != (D,) or top_k > 512:
        return _host_topk(keys, query, actions, top_k)

    keys8 = keys.astype(F8)
    qa = _qarr(query)

    # pass 1: partial fp8 dots (d < 1536) for all rows
    dots1 = _run(_get_nc_p1(), _make_in_maps_p1(keys8, qa))[:N]
    # screen to M1 survivors by partial dot
    cand1 = np.argpartition(-dots1, M1 - 1)[:M1]
    # pass 2: remaining 512 dims for survivors only
    dots2 = _run(_get_nc_p2(), _make_in_maps_p2(keys8, qa, cand1))
    full8 = dots1[cand1] + dots2

    # final top-CAND by full fp8 dot, exact fp32 cosine re-score
    m = min(max(CAND, 4 * top_k), M1)
    sel = np.argpartition(-full8, m - 1)[:m]
    cand = cand1[sel]
    kc = keys[cand]
    d_ex = kc @ query
    n_ex = np.sqrt((kc * kc).sum(axis=1))
    q_norm = np.float32(np.linalg.norm(query))
    sims_c = d_ex / np.maximum(n_ex * q_norm, np.float32(EPS))

    order = np.lexsort((cand, -sims_c))
    idx = cand[order[:top_k]]
    return actions[idx]
